# revision 27
# baseline (speedup 1.0000x reference)
"""Adaptive per-pixel LoG 9x9 convolution on 8 TRN2 NeuronCores.

out[b,c,y,x] = sum_{dy,dx in [-4,4]} xpad[b,c,y+dy,x+dx] * K(dx^2+dy^2; p)
K depends on the offset only through r2 = dx^2+dy^2 (15 distinct values)
-> exact rank-15 decomposition  out = sum_v Gp_v * S_v  where S_v are
fixed ring-sum convolutions and Gp_v are host-computed per-pixel weight
planes.

Row-partition layout: 8 cores = 4 batches x 2 row-halves; partition p =
image row p of the half (half1 is vertically flipped by the host so one
SPMD program serves all cores; reflect at the image edge is baked into
the stationary matrices, and the 4 bottom rows' taps that fall outside
the 128-row window arrive as a tiny host-computed bias plane).

Engine split (vs. the all-DVE tile-layout baseline):
- DVE: 6 per-channel column-class sum ops (U), per-unit products vs Gp,
  tree-reduce + bias add. ~14us instead of ~28us.
- PE: all row-band/ring accumulation = 25 banded-stationary matmuls per
  (channel, column-half) unit into PSUM (fp32), FD=128, plane-major
  accumulation groups (interleaved groups mis-accumulate), PSUM
  double-buffered 2x4 banks.
- ACT: evacuates each unit's 15 S-planes PSUM->SBUF bf16 (unit 0 split
  in two so the DVE product stream starts earlier).
- Output DMAed per column-half; the last half rides both queues.
"""

import math

import numpy as np

B, C, H, W = 4, 3, 256, 256
PAD = 4
SIGMA_MIN, SIGMA_MAX = 0.5, 10.0
N_CORES = 8
DIAG = math.sqrt(H * H + W * W)

NV = 15
V_ORD = [1, 4, 9, 16, 5, 10, 17, 13, 20, 25, 2, 8, 18, 32]  # + center v=0 at 14

# plane -> [(d=|dy| class, acl)], acl: 0..3 = |dx| 1..4, 4 = center col
PLANE_TERMS = [
    [(0, 0), (1, 4)],   # v1
    [(0, 1), (2, 4)],   # v4
    [(0, 2), (3, 4)],   # v9
    [(0, 3), (4, 4)],   # v16
    [(1, 1), (2, 0)],   # v5
    [(1, 2), (3, 0)],   # v10
    [(1, 3), (4, 0)],   # v17
    [(2, 2), (3, 1)],   # v13
    [(2, 3), (4, 1)],   # v20
    [(3, 3), (4, 2)],   # v25
    [(1, 0)],           # v2
    [(2, 1)],           # v8
    [(3, 2)],           # v18
    [(4, 3)],           # v32
    [(0, 4)],           # v0 center
]
ACL_DX = [[-1, 1], [-2, 2], [-3, 3], [-4, 4], [0]]

# plane-major matmul emission order: each plane's accumulation group is
# consecutive (interleaved start/stop groups mis-accumulate on HW)
_EMIT = []
for _p, _terms in enumerate(PLANE_TERMS):
    for _i, (_td, _ta) in enumerate(sorted(_terms)):
        _EMIT.append((_p, _ta, _td, _i == 0, _i == len(_terms) - 1))
assert len(_EMIT) == 25
# index of the matmul that completes plane 7 (planes 0..7 evacuate first
# for unit 0)
_MID = sum(len(PLANE_TERMS[p]) for p in range(8)) - 1  # = 15

XP_N = C * 264         # 792
G_FLAT = NV * 256      # 3840
BST_FLAT = 5 * 128     # 640
O_FLAT = 2 * C * 128   # 768


def _build_program(nc, bass, mybir):
    bf16 = mybir.dt.bfloat16
    f32 = mybir.dt.float32
    Alu = mybir.AluOpType
    Act = mybir.ActivationFunctionType

    xall_d = nc.declare_dram_parameter("xall", [128, XP_N], bf16, isOutput=False)
    g_d = nc.declare_dram_parameter("g", [128, G_FLAT], bf16, isOutput=False)
    bst_d = nc.declare_dram_parameter("bst", [128, BST_FLAT], bf16, isOutput=False)
    bias_d = nc.declare_dram_parameter("bias", [128, O_FLAT], bf16, isOutput=False)
    out_d = nc.declare_dram_parameter("out", [128, O_FLAT], bf16, isOutput=True)

    xa_sem = nc.alloc_semaphore("xa_sem")      # xp (sync queue)
    bst_sem = nc.alloc_semaphore("bst_sem")
    ga_sem = nc.alloc_semaphore("ga_sem")
    gb_sem = nc.alloc_semaphore("gb_sem")
    bis_sem = nc.alloc_semaphore("bis_sem")
    u_sem = nc.alloc_semaphore("u_sem")
    pe_mid_sem = nc.alloc_semaphore("pe_mid_sem")
    pe_sem = nc.alloc_semaphore("pe_sem")
    act_sem = nc.alloc_semaphore("act_sem")
    dh0_sem = nc.alloc_semaphore("dh0_sem")
    dh1_sem = nc.alloc_semaphore("dh1_sem")
    od0_sem = nc.alloc_semaphore("od0_sem")
    od1_sem = nc.alloc_semaphore("od1_sem")

    xall = nc.alloc_sbuf_tensor("s_xall", [128, XP_N], bf16)
    U = nc.alloc_sbuf_tensor("U", [128, 4, C, 256], bf16)
    Ssb = nc.alloc_sbuf_tensor("Ssb", [128, NV, C, 256], bf16)
    # P plane 15 holds the host bias -> 16-term binary tree reduce
    P = nc.alloc_sbuf_tensor("P", [128, 16, C, 256], bf16)
    G = nc.alloc_sbuf_tensor("G", [128, 2, NV, 128], bf16)
    BST = nc.alloc_sbuf_tensor("BST", [128, 5, 128], bf16)
    O = nc.alloc_sbuf_tensor("O", [128, 2, C, 128], bf16)
    scratch = nc.alloc_sbuf_tensor("scratch", [128, 2], bf16)

    ps = [
        nc.alloc_psum_tensor("ps0", [128, NV, 128], f32),
        nc.alloc_psum_tensor("ps1", [128, NV, 128], f32),
    ]

    GA = 1920

    with nc.Block(no_gpsimd_drain=True) as block:
        def flat(t, lo, hi):
            a = t[:]
            return bass.AP(t, lo, [list(a.ap[0]), [1, hi - lo]])

        pd_xall = None  # filled below

        @block.sync
        def _(sync):
            sync.dma_start(out=flat(xall, 0, XP_N), in_=xall_d[:]).then_inc(
                xa_sem, 16
            )
            # G half0 (cols 0..127) gates the first products
            sync.dma_start(out=flat(G, 0, GA), in_=g_d[:, 0:GA]).then_inc(
                gb_sem, 16
            )
            sync.wait_ge(dh0_sem, 1)
            sync.dma_start(out=out_d[:, 0:384], in_=flat(O, 0, 384)).then_inc(
                od0_sem, 16
            )
            sync.wait_ge(dh1_sem, 1)
            sync.dma_start(out=out_d[:, 384:576], in_=flat(O, 384, 576)).then_inc(
                od1_sem, 16
            )
            sync.wait_ge(od0_sem, 16)
            sync.wait_ge(od1_sem, 32)

        @block.gpsimd
        def _(gpsimd):
            gpsimd.wait_ge(od0_sem, 16)
            gpsimd.wait_ge(od1_sem, 32)

        @block.scalar
        def _(scalar):
            # bst first: it gates the PE start and is small; slot-2 spots
            # showed multi-microsecond arrival variance across cores
            scalar.dma_start(out=flat(BST, 0, BST_FLAT), in_=bst_d[:]).then_inc(
                bst_sem, 16
            )
            # G half1 — needed only from products unit 3
            scalar.dma_start(out=flat(G, GA, G_FLAT), in_=g_d[:, GA:]).then_inc(
                ga_sem, 16
            )
            # bias lands directly in P plane 15 (the 16th tree term)
            scalar.dma_start(
                out=flat(P, 15 * C * 256, 16 * C * 256), in_=bias_d[:]
            ).then_inc(bis_sem, 16)
            # dummy op hoists the lazy ACT_TABLE_LOAD off the critical path
            zero_ap = nc.const_aps.aps[(mybir.dt.float32, 0.0)]
            scalar.activation(scratch[:, 0:1], zero_ap, Act.Copy)
            scalar.wait_ge(pe_mid_sem, 1)
            scalar.activation(
                Ssb[:, 0:8, 0, 0:128], ps[0][:, 0:8], Act.Copy
            ).then_inc(act_sem, 1)
            scalar.wait_ge(pe_sem, 1)
            scalar.activation(
                Ssb[:, 8:NV, 0, 0:128], ps[0][:, 8:NV], Act.Copy
            ).then_inc(act_sem, 1)
            for u in range(1, 6):
                h, c = divmod(u, 3)
                hs = 128 * h
                scalar.wait_ge(pe_sem, u + 1)
                scalar.activation(
                    Ssb[:, :, c, hs:hs + 128], ps[u % 2][:], Act.Copy
                ).then_inc(act_sem, 1)
            scalar.wait_ge(dh1_sem, 1)
            scalar.dma_start(out=out_d[:, 576:768], in_=flat(O, 576, 768)).then_inc(
                od1_sem, 16
            )
            scalar.wait_ge(od1_sem, 32)

        @block.tensor
        def _(tensor):
            pd = list(xall[:].ap[0])
            # dummy matmuls prime the LDWEIGHTS/matmul pipeline while the
            # input DMAs are still in flight (unit 0 otherwise runs ~2x
            # slower). They read whatever is in SBUF (garbage is fine) and
            # ps[1] is re-zeroed by unit 1's start=True matmuls.
            for _ in range(6):
                tensor.matmul(
                    ps[1][:, 0], BST[:, 0], BST[:, 0],
                    start=True, stop=True, skip_group_check=True,
                )
            tensor.wait_ge(bst_sem, 16)
            tensor.wait_ge(xa_sem, 16)
            for u in range(6):
                h, c = divmod(u, 3)
                hs = 128 * h
                tensor.wait_ge(u_sem, c + 1)
                if u >= 2:
                    tensor.wait_ge(act_sem, u)
                for i, (plane, acl, d, st, sp) in enumerate(_EMIT):
                    if acl == 4:
                        rhs = bass.AP(xall, c * 264 + 4 + hs, [pd, [1, 128]])
                    else:
                        rhs = U[:, acl, c, hs:hs + 128]
                    mm = tensor.matmul(
                        ps[u % 2][:, plane],
                        BST[:, d],
                        rhs,
                        start=st,
                        stop=sp,
                        skip_group_check=True,
                    )
                    if u == 0 and i == _MID:
                        mm.then_inc(pe_mid_sem, 1)
                    if i == len(_EMIT) - 1:
                        mm.then_inc(pe_sem, 1)

        @block.vector
        def _(vector):
            pd = list(xall[:].ap[0])
            pdU = list(U[:].ap[0])

            # U column-class sums, per channel: even classes {1,3} hit
            # DVE 2x mode; odd classes {0,2} run at 1x (odd element
            # offsets) — still cheaper than shipping an aligned copy
            vector.wait_ge(xa_sem, 16)
            for c in range(C):
                vector.tensor_tensor(
                    bass.AP(U, 768 + c * 256, [pdU, [1536, 2], [1, 256]]),
                    bass.AP(xall, c * 264 + 2, [pd, [-2, 2], [1, 256]]),
                    bass.AP(xall, c * 264 + 6, [pd, [2, 2], [1, 256]]),
                    Alu.add,
                )
                vector.tensor_tensor(
                    bass.AP(U, c * 256, [pdU, [1536, 2], [1, 256]]),
                    bass.AP(xall, c * 264 + 3, [pd, [-2, 2], [1, 256]]),
                    bass.AP(xall, c * 264 + 5, [pd, [2, 2], [1, 256]]),
                    Alu.add,
                ).then_inc(u_sem, 1)

            def products(lo, hi, c, h):
                hs = 128 * h
                vector.tensor_tensor(
                    P[:, lo:hi, c, hs:hs + 128],
                    Ssb[:, lo:hi, c, hs:hs + 128],
                    G[:, h, lo:hi, :],
                    Alu.mult,
                )

            # 16-term binary tree: 15 products + the bias plane (P[15])
            def tree_l1(h, cs):
                hs = 128 * h
                vector.tensor_tensor(
                    P[:, 0:8, cs, hs:hs + 128],
                    P[:, 0:8, cs, hs:hs + 128],
                    P[:, 8:16, cs, hs:hs + 128],
                    Alu.add,
                )

            def tree_rest(h):
                hs = 128 * h
                vector.tensor_tensor(
                    P[:, 0:4, :, hs:hs + 128],
                    P[:, 0:4, :, hs:hs + 128],
                    P[:, 4:8, :, hs:hs + 128],
                    Alu.add,
                )
                vector.tensor_tensor(
                    P[:, 0:2, :, hs:hs + 128],
                    P[:, 0:2, :, hs:hs + 128],
                    P[:, 2:4, :, hs:hs + 128],
                    Alu.add,
                )
                return vector.tensor_tensor(
                    O[:, h], P[:, 0, :, hs:hs + 128], P[:, 1, :, hs:hs + 128],
                    Alu.add,
                )

            vector.wait_ge(gb_sem, 16)
            vector.wait_ge(act_sem, 1)
            products(0, 8, 0, 0)
            vector.wait_ge(act_sem, 2)
            products(8, NV, 0, 0)
            for u in range(1, 6):
                h, c = divmod(u, 3)
                vector.wait_ge(act_sem, u + 2)
                if u == 3:
                    vector.wait_ge(ga_sem, 16)
                products(0, NV, c, h)
                if u == 2:
                    vector.wait_ge(bis_sem, 16)
                    tree_l1(0, slice(0, C))
                    tree_rest(0).then_inc(dh0_sem, 1)
                elif u == 5:
                    tree_l1(1, slice(0, C))
                    tree_rest(1).then_inc(dh1_sem, 1)

    return nc


_PROGRAM_CACHE = {}


def _get_program():
    if "nc" not in _PROGRAM_CACHE:
        import sys

        if "/opt/trn_rl_repo" not in sys.path:
            sys.path.insert(0, "/opt/trn_rl_repo")
        from concourse import bass, mybir

        nc = bass.Bass()
        _PROGRAM_CACHE["nc"] = _build_program(nc, bass, mybir)
    return _PROGRAM_CACHE["nc"]


def _build_bst():
    bst = np.zeros((5, 128, 128), np.float32)  # [d, in row i, out row r]
    for d in range(5):
        for r in range(128):
            for s in ({d, -d} if d else {0}):
                i = r + s
                if i < 0:
                    i = -i  # top reflect
                if i <= 127:
                    bst[d, i, r] += 1.0
    return bst


def _host_prep(x, foa_xy):
    import ml_dtypes

    bf = ml_dtypes.bfloat16
    x = np.asarray(x)
    bst = _build_bst().transpose(1, 0, 2)  # [i, d, r]
    bst_flat = np.ascontiguousarray(bst.reshape(128, BST_FLAT).astype(bf))
    in_maps = []
    for core in range(N_CORES):
        b, half = divmod(core, 2)
        xb = x[b] if half == 0 else x[b][:, ::-1, :]
        xw = xb[:, 0:132, :]
        xpad = np.pad(xw, ((0, 0), (0, 0), (PAD, PAD)), mode="reflect")  # [3,132,264]
        xp = np.ascontiguousarray(xpad[:, 0:128, :].transpose(1, 0, 2)).astype(bf)
        xall = xp.reshape(128, XP_N)

        rp = np.arange(128)
        yy_img = rp if half == 0 else 255 - rp
        yy, xx = np.meshgrid(
            yy_img.astype(np.float64), np.arange(W, dtype=np.float64), indexing="ij"
        )
        fx, fy = float(foa_xy[b, 0]), float(foa_xy[b, 1])
        dist = np.sqrt((xx - fx) ** 2 + (yy - fy) ** 2)
        dn = dist / DIAG
        sigma = (1.0 - dn) * SIGMA_MIN + dn * SIGMA_MAX
        inv2s2 = 1.0 / (2.0 * sigma * sigma)
        base = -dist * np.sqrt(sigma) / (math.pi * sigma ** 4)
        Gf = np.empty((128, NV, 256), np.float32)
        for i, v in enumerate(V_ORD):
            t = v * inv2s2
            Gf[:, i] = base * (1.0 - t) * np.exp(-t)
        Gf[:, 14] = base

        # bias for out rows 124..127: taps at rows 128..131 (outside window)
        rows = xpad[:, 128:132, :].astype(np.float32)  # [3, 4, 264]
        cs = np.zeros((5, 4, C, 256), np.float32)
        for a in range(5):
            for dx in ACL_DX[a]:
                cs[a] += rows[:, :, 4 + dx:4 + dx + 256].transpose(1, 0, 2)
        bias = np.zeros((128, C, 256), np.float32)
        for plane, terms in enumerate(PLANE_TERMS):
            for (d, acl) in terms:
                if d == 0:
                    continue
                for r in range(124, 128):
                    i = r + d
                    if i >= 128:
                        bias[r] += Gf[r, plane][None, :] * cs[acl, i - 128]
        bias_t = np.ascontiguousarray(bias).astype(bf)  # [128, C, 256]
        # G in column-half-major layout [128, 2, 15, 128]
        g_t = np.ascontiguousarray(
            Gf.astype(bf).reshape(128, NV, 2, 128).transpose(0, 2, 1, 3)
        )

        in_maps.append(
            {
                "xall": np.ascontiguousarray(xall),
                "g": g_t.reshape(128, G_FLAT),
                "bst": bst_flat,
                "bias": bias_t.reshape(128, O_FLAT),
            }
        )
    return in_maps


def _gather(results):
    out = np.empty((B, C, H, W), dtype=np.float32)
    for core in range(N_CORES):
        b, half = divmod(core, 2)
        o = results[core]["out"].astype(np.float32).reshape(128, 2, C, 128)
        o = o.transpose(2, 0, 1, 3).reshape(C, 128, 256)
        if half:
            o = o[:, ::-1, :]
        out[b, :, half * 128:half * 128 + 128, :] = o
    return out


def kernel(x, foa_xy, _trace=False, _tmpdir=None):
    import sys

    if "/opt/trn_rl_repo" not in sys.path:
        sys.path.insert(0, "/opt/trn_rl_repo")
    from concourse.bass_utils import run_bass_kernel_spmd

    nc = _get_program()
    in_maps = _host_prep(np.asarray(x), np.asarray(foa_xy))
    kw = {}
    if _trace:
        kw = dict(trace=True, trace_cores=[], tmpdir=_tmpdir)
    res = run_bass_kernel_spmd(nc, in_maps, list(range(N_CORES)), **kw)
    out = _gather(res.results)
    if _trace:
        return out, res
    return out
